# revision 29
# baseline (speedup 1.0000x reference)
"""Trainium2 Bass kernel: 3x3 conv (N=16, C_in=16, C_out=64, H=W=256, pad=1).

Strategy (8 NeuronCores, data-parallel over batch N -> 2 images/core):
  - Host pads x to [2,16,258,258] (zero ring) so the kernel has no edge cases.
  - Per 64-row "superstep": two 32-row strips (A,B) are stacked on SBUF
    partitions 0-47 / 48-95 as (kh, ci) im2col slabs; kh-shifted blocks are
    built with two SBUF->SBUF DMA copies from the center block.
  - One matmul per kw tap (3 total, PSUM-accumulated) with a [96,128]
    block-diagonal fp32r weight matrix computes both strips' 64 output
    channels for 512 pixels (2 rows x 256) in one instruction; kw shifts are
    pure free-dim offsets into the 258-pitch slab (gap columns are zero).
  - PSUM -> SBUF evacuation on VectorE, 512KB store DMAs.
"""

import sys

if "/opt/trn_rl_repo" not in sys.path:
    sys.path.insert(0, "/opt/trn_rl_repo")

import numpy as np

import concourse.bacc as bacc
import concourse.bass as bass
import concourse.mybir as mybir
import concourse.tile as tile
from concourse.bass_utils import run_bass_kernel_spmd

N_FULL, CI, CO, H, W_SP = 16, 16, 64, 256, 256
NCORES = 8
NB = N_FULL // NCORES          # batches per core
HP, WP = H + 2, W_SP + 2       # padded image dims
SLOT = WP                      # 258: one row-slot in the slab (z x0..x255 z)
RSTRIP = 32                    # output rows per strip
SLOTS = RSTRIP + 2             # row-slots per strip slab (rows + 2 halo)
NSS = H // (2 * RSTRIP)        # supersteps per image (4)
NBANK = RSTRIP // 2            # PSUM banks per superstep (16, pool rotates 8)
F32 = mybir.dt.float32
F32R = mybir.dt.float32r

_CACHE = {}


def _build(reps: int = 1):
    nc = bacc.Bacc("TRN2", target_bir_lowering=False, debug=False)
    x_d = nc.dram_tensor("xp", [NB, CI, HP, WP], F32, kind="ExternalInput").ap()
    w_d = nc.dram_tensor("wts", [3, 96, 128], F32, kind="ExternalInput").ap()
    o_d = nc.dram_tensor("out", [NB, CO, H, W_SP], F32, kind="ExternalOutput").ap()

    # out[n, co, (t, s, j, r), w] view for per-(superstep, strip, evac) stores
    o_v = o_d.rearrange("n c (t s j r) w -> n t s j c (r w)", t=NSS, s=2, j=4)

    xe_n = CI * HP * WP        # x_pad element strides
    xe_c = HP * WP
    xe_h = WP

    with tile.TileContext(nc) as tc:
        with (
            tc.tile_pool(name="wp", bufs=1) as wpool,
            tc.tile_pool(name="slab", bufs=4) as slabpool,
            tc.tile_pool(name="evac", bufs=6) as evacpool,
            tc.tile_pool(name="ps", bufs=8, space="PSUM") as pspool,
        ):
            # weights, loaded once (SWDGE cast fp32 -> fp32r)
            wsb = wpool.tile([96, 3 * 128], F32R)
            for kw in range(3):
                nc.gpsimd.dma_start(wsb[:, kw * 128 : (kw + 1) * 128], w_d[kw])

            def build_slab(n, t):
                # slab partition layout: [block0(A,B) | center(A,B) | block2(A,B)]
                # i.e. partition = kh*32 + strip*16 + ci.  Center loads are
                # per-strip (SBUF-side DMA APs must stay 2D single-level);
                # the kh=0 / kh=2 blocks are each ONE row-shifted 32-partition
                # SBUF->SBUF copy of both centers.
                h0 = 2 * RSTRIP * t
                slab = slabpool.tile([96, SLOTS * SLOT], F32R, tag="slab")
                sf = slab[:]
                for strip in range(2):
                    src = bass.AP(
                        x_d.tensor,
                        n * xe_n + (h0 + strip * RSTRIP) * xe_h,
                        [[xe_c, CI], [1, SLOTS * SLOT]],
                    )
                    nc.gpsimd.dma_start(sf[32 + 16 * strip : 48 + 16 * strip, :], src)
                nc.sync.dma_start(
                    sf[0:32, SLOT : (SLOTS - 1) * SLOT],
                    sf[32:64, 0 : (SLOTS - 2) * SLOT],
                )
                nc.sync.dma_start(
                    sf[64:96, SLOT : (SLOTS - 1) * SLOT],
                    sf[32:64, 2 * SLOT : SLOTS * SLOT],
                )
                return slab

            def compute(n, t, slab):
                su = slab[:].rearrange("p (u e) -> p u e", u=SLOTS)
                for j in range(4):
                    evac = evacpool.tile([128, 4 * 512], F32, tag="evac")
                    for bb in range(4):
                        b = 4 * j + bb
                        ps = pspool.tile([128, 512], F32, tag="ps")
                        for kw in range(3):
                            rhs = su[:, 2 * b + 1 : 2 * b + 3, kw : kw + 256]
                            nc.tensor.matmul(
                                ps[:],
                                wsb[:, kw * 128 : (kw + 1) * 128],
                                rhs,
                                start=(kw == 0),
                                stop=(kw == 2),
                            )
                        nc.vector.tensor_copy(
                            evac[:, bb * 512 : (bb + 1) * 512], ps[:]
                        )
                    for strip in range(2):
                        nc.sync.dma_start(
                            o_v[n, t, strip, j],
                            evac[strip * 64 : (strip + 1) * 64, :],
                        )

            # software pipeline with two-superstep lookahead on slab builds
            LOOK = 3
            steps = [(n, t) for _ in range(reps) for n in range(NB) for t in range(NSS)]
            slabs = {}
            for k in range(min(LOOK, len(steps))):
                slabs[steps[k]] = build_slab(*steps[k])
            for i, (n, t) in enumerate(steps):
                if i + LOOK < len(steps):
                    slabs[steps[i + LOOK]] = build_slab(*steps[i + LOOK])
                compute(n, t, slabs.pop((n, t)))

    nc.compile()
    return nc


def _prep_weights(W: np.ndarray) -> np.ndarray:
    # lhsT[kw][kh*32 + strip*16 + ci, strip*64 + co] = W[co, ci, kh, kw]
    wts = np.zeros((3, 96, 128), dtype=np.float32)
    blk = np.ascontiguousarray(W.transpose(3, 2, 1, 0))  # [kw, kh, ci, co]
    for kh in range(3):
        for strip in range(2):
            wts[:, kh * 32 + strip * 16 : kh * 32 + (strip + 1) * 16,
                strip * 64 : (strip + 1) * 64] = blk[:, kh]
    return wts


def kernel(x: np.ndarray, W: np.ndarray) -> np.ndarray:
    assert x.shape == (N_FULL, CI, H, W_SP) and W.shape == (CO, CI, 3, 3)
    # BASS_TRACE without the axon NTFF hook module would crash the run path;
    # disable tracing only when the hook is genuinely unavailable.
    try:
        import antenv.axon_hooks  # noqa: F401
    except Exception:
        import os

        os.environ.setdefault("BASS_NEVER_TRACE", "1")
    if "nc" not in _CACHE:
        _CACHE["nc"] = _build()
    nc = _CACHE["nc"]

    wts = _prep_weights(np.asarray(W, dtype=np.float32))
    xs = np.asarray(x, dtype=np.float32).reshape(NCORES, NB, CI, H, W_SP)
    in_maps = []
    for i in range(NCORES):
        xp = np.zeros((NB, CI, HP, WP), dtype=np.float32)
        xp[:, :, 1 : H + 1, 1 : W_SP + 1] = xs[i]
        in_maps.append({"xp": xp, "wts": wts})

    res = run_bass_kernel_spmd(nc, in_maps, list(range(NCORES)))
    out = np.concatenate([res.results[i]["out"] for i in range(NCORES)], axis=0)
    return out


# revision 31
# speedup vs baseline: 1.1161x; 1.1161x over previous
"""Trainium2 Bass kernel: 3x3 conv (N=16, C_in=16, C_out=64, H=W=256, pad=1).

Strategy (8 NeuronCores, data-parallel over batch N -> 2 images/core):
  - Host pads x to [2,16,258,258] (zero ring) so the kernel has no edge cases.
  - Per 64-row "superstep": two 32-row strips (A,B) are stacked on SBUF
    partitions 0-47 / 48-95 as (kh, ci) im2col slabs; kh-shifted blocks are
    built with two SBUF->SBUF DMA copies from the center block.
  - One matmul per kw tap (3 total, PSUM-accumulated) with a [96,128]
    block-diagonal fp32r weight matrix computes both strips' 64 output
    channels for 512 pixels (2 rows x 256) in one instruction; kw shifts are
    pure free-dim offsets into the 258-pitch slab (gap columns are zero).
  - PSUM -> SBUF evacuation on VectorE, 512KB store DMAs.
"""

import sys

if "/opt/trn_rl_repo" not in sys.path:
    sys.path.insert(0, "/opt/trn_rl_repo")

import numpy as np

import concourse.bacc as bacc
import concourse.bass as bass
import concourse.mybir as mybir
import concourse.tile as tile
from concourse.bass_utils import run_bass_kernel_spmd

N_FULL, CI, CO, H, W_SP = 16, 16, 64, 256, 256
NCORES = 8
NB = N_FULL // NCORES          # batches per core
HP, WP = H + 2, W_SP + 2       # padded image dims
SLOT = WP                      # 258: one row-slot in the slab (z x0..x255 z)
RSTRIP = 32                    # output rows per strip
SLOTS = RSTRIP + 2             # row-slots per strip slab (rows + 2 halo)
NSS = H // (2 * RSTRIP)        # supersteps per image (4)
NBANK = RSTRIP // 2            # PSUM banks per superstep (16, pool rotates 8)
F32 = mybir.dt.float32
F32R = mybir.dt.float32r

_CACHE = {}


def _build(reps: int = 1):
    nc = bacc.Bacc("TRN2", target_bir_lowering=False, debug=False)
    x_d = nc.dram_tensor("xp", [NB, CI, HP, WP], F32, kind="ExternalInput").ap()
    w_d = nc.dram_tensor("wts", [3, 96, 128], F32, kind="ExternalInput").ap()
    o_d = nc.dram_tensor("out", [NB, CO, H, W_SP], F32, kind="ExternalOutput").ap()

    # out[n, co, (t, s, j, r), w] view for per-(superstep, strip, evac) stores
    o_v = o_d.rearrange("n c (t s j r) w -> n t s j c (r w)", t=NSS, s=2, j=4)

    xe_n = CI * HP * WP        # x_pad element strides
    xe_c = HP * WP
    xe_h = WP

    with tile.TileContext(nc) as tc:
        with (
            tc.tile_pool(name="wp", bufs=1) as wpool,
            tc.tile_pool(name="slab", bufs=4) as slabpool,
            tc.tile_pool(name="evac", bufs=6) as evacpool,
            tc.tile_pool(name="ps", bufs=8, space="PSUM") as pspool,
        ):
            # weights, loaded once (SWDGE cast fp32 -> fp32r)
            wsb = wpool.tile([96, 3 * 128], F32R)
            for kw in range(3):
                nc.gpsimd.dma_start(wsb[:, kw * 128 : (kw + 1) * 128], w_d[kw])

            def build_slab(n, t):
                # slab partition layout: [block0(A,B) | center(A,B) | block2(A,B)]
                # i.e. partition = kh*32 + strip*16 + ci.  Center loads are
                # per-strip (SBUF-side DMA APs must stay 2D single-level);
                # the kh=0 / kh=2 blocks are each ONE row-shifted 32-partition
                # SBUF->SBUF copy of both centers.
                h0 = 2 * RSTRIP * t
                slab = slabpool.tile([96, SLOTS * SLOT], F32R, tag="slab")
                sf = slab[:]
                for strip in range(2):
                    src = bass.AP(
                        x_d.tensor,
                        n * xe_n + (h0 + strip * RSTRIP) * xe_h,
                        [[xe_c, CI], [1, SLOTS * SLOT]],
                    )
                    nc.gpsimd.dma_start(sf[32 + 16 * strip : 48 + 16 * strip, :], src)
                nc.scalar.copy(
                    sf[0:32, SLOT : (SLOTS - 1) * SLOT],
                    sf[32:64, 0 : (SLOTS - 2) * SLOT],
                )
                nc.scalar.copy(
                    sf[64:96, SLOT : (SLOTS - 1) * SLOT],
                    sf[32:64, 2 * SLOT : SLOTS * SLOT],
                )
                return slab

            def compute(n, t, slab):
                su = slab[:].rearrange("p (u e) -> p u e", u=SLOTS)
                for j in range(4):
                    evac = evacpool.tile([128, 4 * 512], F32, tag="evac")
                    for bb in range(4):
                        b = 4 * j + bb
                        ps = pspool.tile([128, 512], F32, tag="ps")
                        for kw in range(3):
                            rhs = su[:, 2 * b + 1 : 2 * b + 3, kw : kw + 256]
                            nc.tensor.matmul(
                                ps[:],
                                wsb[:, kw * 128 : (kw + 1) * 128],
                                rhs,
                                start=(kw == 0),
                                stop=(kw == 2),
                            )
                        nc.vector.tensor_copy(
                            evac[:, bb * 512 : (bb + 1) * 512], ps[:]
                        )
                    for strip in range(2):
                        nc.sync.dma_start(
                            o_v[n, t, strip, j],
                            evac[strip * 64 : (strip + 1) * 64, :],
                        )

            # software pipeline with two-superstep lookahead on slab builds
            LOOK = 3
            steps = [(n, t) for _ in range(reps) for n in range(NB) for t in range(NSS)]
            slabs = {}
            for k in range(min(LOOK, len(steps))):
                slabs[steps[k]] = build_slab(*steps[k])
            for i, (n, t) in enumerate(steps):
                if i + LOOK < len(steps):
                    slabs[steps[i + LOOK]] = build_slab(*steps[i + LOOK])
                compute(n, t, slabs.pop((n, t)))

    nc.compile()
    return nc


def _prep_weights(W: np.ndarray) -> np.ndarray:
    # lhsT[kw][kh*32 + strip*16 + ci, strip*64 + co] = W[co, ci, kh, kw]
    wts = np.zeros((3, 96, 128), dtype=np.float32)
    blk = np.ascontiguousarray(W.transpose(3, 2, 1, 0))  # [kw, kh, ci, co]
    for kh in range(3):
        for strip in range(2):
            wts[:, kh * 32 + strip * 16 : kh * 32 + (strip + 1) * 16,
                strip * 64 : (strip + 1) * 64] = blk[:, kh]
    return wts


def kernel(x: np.ndarray, W: np.ndarray) -> np.ndarray:
    assert x.shape == (N_FULL, CI, H, W_SP) and W.shape == (CO, CI, 3, 3)
    # BASS_TRACE without the axon NTFF hook module would crash the run path;
    # disable tracing only when the hook is genuinely unavailable.
    try:
        import antenv.axon_hooks  # noqa: F401
    except Exception:
        import os

        os.environ.setdefault("BASS_NEVER_TRACE", "1")
    if "nc" not in _CACHE:
        _CACHE["nc"] = _build()
    nc = _CACHE["nc"]

    wts = _prep_weights(np.asarray(W, dtype=np.float32))
    xs = np.asarray(x, dtype=np.float32).reshape(NCORES, NB, CI, H, W_SP)
    in_maps = []
    for i in range(NCORES):
        xp = np.zeros((NB, CI, HP, WP), dtype=np.float32)
        xp[:, :, 1 : H + 1, 1 : W_SP + 1] = xs[i]
        in_maps.append({"xp": xp, "wts": wts})

    res = run_bass_kernel_spmd(nc, in_maps, list(range(NCORES)))
    out = np.concatenate([res.results[i]["out"] for i in range(NCORES)], axis=0)
    return out
